# revision 36
# baseline (speedup 1.0000x reference)
"""Trainium2 Bass kernel for nn_Attention_7945689497706.

v5 structure:
- K=64 row-tiled sim matmul pairs (both heads of a pair concurrently on
  PE row groups 0:64 / 64:128 — no zero padding).
- Attention as one flat 68-round stream per batch: round r issues the
  sim pair of (sweep r//8, jc r%8), the av matmuls of round r-2, and the
  sweep epilogues lagged so no PE instruction heads the queue waiting.
- Weights kept in f32r, DMA'd straight into SBUF: gamma folds into xn
  (scalar_tensor_tensor), the q-scale dh^-0.5 folds into exp's scale.
- Each batch self-fills its q/k/v projections and the other batch's
  work into its own exp-bound attention bubbles as 2-MM units.
"""

import ml_dtypes
import numpy as np

import concourse.bass as bass
import concourse.mybir as mybir
import concourse.tile as tile
from concourse import bacc
from concourse.bass_utils import run_bass_kernel_spmd

F32 = mybir.dt.float32
F32R = mybir.dt.float32r
BF16 = mybir.dt.bfloat16
AF = mybir.ActivationFunctionType

NCORES = 8
B = 16
C = 512
N = 1024          # pixels = 32*32
HEADS = 8
DH = 64
NMEM = 4
PB = B // NCORES  # batch elements per core
CT = C // 128     # channel partition-tiles
NPAIR = HEADS // 2
VW = HEADS * (DH + 1)  # vext width: per head [v | ones] = 65
QSC = DH ** -0.5


def _build():
    nc = bacc.Bacc()
    x_ext = nc.declare_dram_parameter("x", [PB, C, N], BF16, isOutput=False)
    wqkvt_ext = nc.declare_dram_parameter("wqkvt", [C, 3 * C], BF16, isOutput=False)
    wot_ext = nc.declare_dram_parameter("wot", [C, C], BF16, isOutput=False)
    gammat_ext = nc.declare_dram_parameter("gammat", [128, CT], F32, isOutput=False)
    memk_ext = nc.declare_dram_parameter("memk", [128, NPAIR, NMEM], F32, isOutput=False)
    memv_ext = nc.declare_dram_parameter("memv", [128, 2, VW], F32, isOutput=False)
    out_ext = nc.declare_dram_parameter("out", [PB, C, N], BF16, isOutput=True)

    with tile.TileContext(nc) as tc:
        with (
            tc.tile_pool(name="const", bufs=1) as const,
            tc.tile_pool(name="wstage", bufs=1) as wstage,
            tc.tile_pool(name="xp", bufs=2) as xp,
            tc.tile_pool(name="data", bufs=1) as data,
            tc.tile_pool(name="atp", bufs=2) as atp,
            tc.tile_pool(name="qp", bufs=2) as qp,
            tc.tile_pool(name="pp", bufs=4) as pp,
            tc.tile_pool(name="pm", bufs=4) as pm,
            tc.tile_pool(name="avs", bufs=2) as avsp,
            tc.tile_pool(name="rp", bufs=2) as rp,
            tc.tile_pool(name="ob", bufs=3) as obp,
            tc.tile_pool(name="qkv_ps", bufs=2, space="PSUM") as qkv_ps,
            tc.tile_pool(name="sim_ps", bufs=2, space="PSUM") as sim_ps,
            tc.tile_pool(name="av_ps", bufs=2, space="PSUM") as av_ps,
        ):
            # ---------------- DMA staging ----------------
            xraws = []
            for bb in range(PB):
                xr = xp.tile([128, CT, N], BF16, tag="xraw")
                xraws.append(xr)

            wqkv = const.tile([128, CT, 3 * C], BF16, tag="wqkv")
            wo = const.tile([128, CT, C], BF16, tag="wo")
            g1 = const.tile([128, CT], F32, tag="g1")
            ones1 = const.tile([128, 64], F32R, tag="ones1")
            ones128 = const.tile([128, 128], BF16, tag="ones128")
            # kT packed per head-pair: rows 0:64 = even head (d), 64:128 = odd
            kTp = const.tile([128, NPAIR, 1028], BF16, tag="kTp")
            vextA = const.tile([128, 8, VW], BF16, tag="vextA")
            vextB = const.tile([128, 8, VW], BF16, tag="vextB")
            vmem = const.tile([128, 2, VW], BF16, tag="vmem")
            gsb = const.tile([128, CT], F32, tag="gsb")
            vexts = [vextA, vextB]

            # x0 split across all four DMA queues so norm-0 starts asap
            xq = [nc.sync, nc.scalar, nc.gpsimd, nc.sync]
            for t in range(CT):
                xq[t].dma_start(out=xraws[0][:, t, :], in_=x_ext[0, t * 128:(t + 1) * 128, :])
            nc.scalar.dma_start(out=gsb, in_=gammat_ext[:, :])
            for t in range(CT):
                (nc.sync if t < 2 else nc.scalar).dma_start(
                    out=wqkv[:, t, :], in_=wqkvt_ext[t * 128:(t + 1) * 128, :])
            for t in range(CT):
                nc.gpsimd.dma_start(
                    out=xraws[1][:, t, :], in_=x_ext[1, t * 128:(t + 1) * 128, :])
            ws = wstage.tile([128, NPAIR * NMEM + 2 * VW], F32, tag="ws")
            nc.gpsimd.dma_start(out=ws[:, 0:NPAIR * NMEM],
                                in_=memk_ext[:, :, :].rearrange("p g c -> p (g c)"))
            nc.gpsimd.dma_start(out=ws[:, NPAIR * NMEM:NPAIR * NMEM + 2 * VW],
                                in_=memv_ext[:, :, :].rearrange("p g c -> p (g c)"))
            # out-proj weights are only needed late; keep them last on sync
            for t in range(CT):
                nc.sync.dma_start(out=wo[:, t, :], in_=wot_ext[t * 128:(t + 1) * 128, :])

            nc.scalar.activation(out=g1, in_=gsb, func=AF.Copy, bias=1.0)
            nc.vector.memset(ones128, 1.0)
            # HAM warm-up: ~40 dependency-free matmuls so the PE clock gate
            # opens (1.2 -> 2.4 GHz) before the real prologue chains arrive.
            warm_ps = qkv_ps.tile([128, 512], F32, tag="q")
            for _ in range(40):
                nc.tensor.matmul(warm_ps[:, 0:128], ones128, ones128,
                                 start=True, stop=True)
            nc.vector.tensor_copy(out=gsb, in_=warm_ps[0:128, 0:CT])
            nc.vector.memset(ones1.bitcast(F32), 1.0)
            nc.vector.tensor_copy(
                out=kTp[:, :, 1024:1028],
                in_=ws[:, 0:NPAIR * NMEM].rearrange("p (g c) -> p g c", c=NMEM))
            nc.vector.tensor_copy(
                out=vmem,
                in_=ws[:, NPAIR * NMEM:NPAIR * NMEM + 2 * VW].rearrange("p (g c) -> p g c", c=VW))
            for v in vexts:
                oc = v[:, :, :].rearrange("p j (h c) -> p j h c", c=DH + 1)[:, :, :, DH:DH + 1]
                nc.gpsimd.memset(oc, 1.0)

            # ---------------- pipeline stages ----------------
            def norm(bb):
                """x -> xn = x * (gamma+1) / rms(x), as schedulable units."""
                xraw = xraws[bb]
                xsq = data.tile([128, CT, N], BF16, tag="xsq")
                xn = data.tile([128, CT, N], BF16, tag="xn" + str(bb))
                st8 = {}

                def xsq_unit(t):
                    def u():
                        if bb == 1 or t < 2:
                            nc.scalar.activation(out=xsq[:, t, :], in_=xraw[:, t, :],
                                                 func=AF.Square)
                        else:
                            nc.vector.tensor_mul(out=xsq[:, t, :], in0=xraw[:, t, :],
                                                 in1=xraw[:, t, :])
                    return u

                def ss_unit(h2, half):
                    def u():
                        if "ss" not in st8:
                            ss = sim_ps.tile([128, N], F32, tag="sim")
                            st8["ss"] = ss
                        ss = st8["ss"]
                        for t in ((0, 1) if half == 0 else (2, 3)):
                            nc.tensor.matmul(ss[:, h2 * 512:(h2 + 1) * 512], ones128,
                                             xsq[:, t, h2 * 512:(h2 + 1) * 512],
                                             start=(t == 0), stop=(t == CT - 1))
                    return u

                def finish_unit():
                    def u():
                        ss = st8["ss"]
                        sroot = data.tile([128, N], F32, tag="sroot")
                        nc.scalar.activation(out=sroot, in_=ss, func=AF.Sqrt, scale=1.0 / C)
                        snorm = data.tile([128, N], F32, tag="snorm")
                        nc.vector.reciprocal_approx_fast(out=snorm, in_=sroot)
                        st8["snorm"] = snorm
                    return u

                def xn_unit(t, h2=None):
                    sl = slice(0, N) if h2 is None else slice(h2 * 512, (h2 + 1) * 512)

                    def u():
                        nc.vector.scalar_tensor_tensor(
                            out=xn[:, t, sl], in0=xraw[:, t, sl], scalar=g1[:, t:t + 1],
                            in1=st8["snorm"][:, sl], op0=mybir.AluOpType.mult,
                            op1=mybir.AluOpType.mult)
                    return u

                if bb == 0:
                    # h2=0 halves first so the first projection chains start
                    # after half the xn work
                    xn_units = ([xn_unit(t, 0) for t in range(CT)]
                                + [xn_unit(t, 1) for t in range(CT)])
                else:
                    xn_units = [xn_unit(t) for t in range(CT)]
                units = ([xsq_unit(t) for t in range(CT)]
                         + [ss_unit(h2, half) for h2 in range(2) for half in range(2)]
                         + [finish_unit()]
                         + xn_units)
                return xn, units

            def qkproj(xn, qT, mcs, h2s=(0, 1)):
                for mc in mcs:
                    for h2 in h2s:
                        ps = qkv_ps.tile([128, 512], F32, tag="q")
                        for t in range(CT):
                            nc.tensor.matmul(ps, wqkv[:, t, mc * 128:(mc + 1) * 128],
                                             xn[:, t, h2 * 512:(h2 + 1) * 512],
                                             start=(t == 0), stop=(t == CT - 1))
                        if mc < 4:
                            nc.vector.tensor_copy(out=qT[:, mc, h2 * 512:(h2 + 1) * 512], in_=ps)
                        else:
                            nc.vector.tensor_copy(
                                out=kTp[:, mc - 4, h2 * 512:(h2 + 1) * 512], in_=ps)

            def qkproj_units(xn, qT, mc):
                state = {}
                units = []
                for h2 in range(2):
                    def u1(h2=h2):
                        ps = qkv_ps.tile([128, 512], F32, tag="q")
                        state[h2] = ps
                        for t in (0, 1):
                            nc.tensor.matmul(ps, wqkv[:, t, mc * 128:(mc + 1) * 128],
                                             xn[:, t, h2 * 512:(h2 + 1) * 512],
                                             start=(t == 0), stop=False)

                    def u2(h2=h2):
                        ps = state[h2]
                        for t in (2, 3):
                            nc.tensor.matmul(ps, wqkv[:, t, mc * 128:(mc + 1) * 128],
                                             xn[:, t, h2 * 512:(h2 + 1) * 512],
                                             start=False, stop=(t == CT - 1))
                        if mc < 4:
                            nc.vector.tensor_copy(out=qT[:, mc, h2 * 512:(h2 + 1) * 512], in_=ps)
                        else:
                            nc.vector.tensor_copy(
                                out=kTp[:, mc - 4, h2 * 512:(h2 + 1) * 512], in_=ps)
                    units.append(u1)
                    units.append(u2)
                return units

            def vproj(xn, vext, ics):
                for ic in ics:
                    ps = qkv_ps.tile([128, 512], F32, tag="q")
                    for t in range(CT):
                        nc.tensor.matmul(ps, xn[:, t, ic * 128:(ic + 1) * 128],
                                         wqkv[:, t, 2 * C:3 * C],
                                         start=(t == 0), stop=(t == CT - 1))
                    ps_h = ps[:, :].rearrange("p (h c) -> p h c", c=DH)
                    vdst = vext[:, ic, :].rearrange("p (h c) -> p h c", c=DH + 1)[:, :, 0:DH]
                    nc.vector.tensor_copy(out=vdst, in_=ps_h)

            def vproj_units(xn, vext, ic):
                state = {}

                def u1():
                    ps = qkv_ps.tile([128, 512], F32, tag="q")
                    state[0] = ps
                    for t in (0, 1):
                        nc.tensor.matmul(ps, xn[:, t, ic * 128:(ic + 1) * 128],
                                         wqkv[:, t, 2 * C:3 * C],
                                         start=(t == 0), stop=False)

                def u2():
                    ps = state[0]
                    for t in (2, 3):
                        nc.tensor.matmul(ps, xn[:, t, ic * 128:(ic + 1) * 128],
                                         wqkv[:, t, 2 * C:3 * C],
                                         start=False, stop=(t == CT - 1))
                    ps_h = ps[:, :].rearrange("p (h c) -> p h c", c=DH)
                    vdst = vext[:, ic, :].rearrange("p (h c) -> p h c", c=DH + 1)[:, :, 0:DH]
                    nc.vector.tensor_copy(out=vdst, in_=ps_h)
                return [u1, u2]

            def proj(attn, bb, mcs=None, h2s=(0, 1)):
                for mc in (range(CT) if mcs is None else mcs):
                    for h2 in h2s:
                        ps = qkv_ps.tile([128, 512], F32, tag="q")
                        for t in range(CT):
                            nc.tensor.matmul(ps, wo[:, t, mc * 128:(mc + 1) * 128],
                                             attn[:, t, h2 * 512:(h2 + 1) * 512],
                                             start=(t == 0), stop=(t == CT - 1))
                        ob = obp.tile([128, 512], BF16, tag="ob")
                        nc.vector.tensor_copy(out=ob, in_=ps)
                        [nc.sync, nc.scalar, nc.gpsimd][(2 * mc + h2) % 3].dma_start(
                            out=out_ext[bb, mc * 128:(mc + 1) * 128, h2 * 512:(h2 + 1) * 512],
                            in_=ob)

            def proj_units(attn, bb, mc, h2):
                state = {}

                def u1():
                    ps = qkv_ps.tile([128, 512], F32, tag="q")
                    state[0] = ps
                    for t in (0, 1):
                        nc.tensor.matmul(ps, wo[:, t, mc * 128:(mc + 1) * 128],
                                         attn[:, t, h2 * 512:(h2 + 1) * 512],
                                         start=(t == 0), stop=False)

                def u2():
                    ps = state[0]
                    for t in (2, 3):
                        nc.tensor.matmul(ps, wo[:, t, mc * 128:(mc + 1) * 128],
                                         attn[:, t, h2 * 512:(h2 + 1) * 512],
                                         start=False, stop=(t == CT - 1))
                    ob = obp.tile([128, 512], BF16, tag="ob")
                    nc.vector.tensor_copy(out=ob, in_=ps)
                    [nc.sync, nc.scalar, nc.gpsimd][(2 * mc + h2) % 3].dma_start(
                        out=out_ext[bb, mc * 128:(mc + 1) * 128, h2 * 512:(h2 + 1) * 512],
                        in_=ob)
                return [u1, u2]

            def memsim(qT, pmems, g):
                st = sim_ps.tile([128, N], F32, tag="sim")
                for h4 in range(4):
                    h = 4 * g + h4
                    p, hh = h // 2, h % 2
                    for h2 in range(2):
                        nc.tensor.matmul(
                            st[32 * h4:32 * h4 + NMEM, h2 * 512:(h2 + 1) * 512],
                            kTp[64 * hh:64 * hh + 64, p, 1024:1028],
                            qT[64 * hh:64 * hh + 64, p, h2 * 512:(h2 + 1) * 512],
                            start=True, stop=True, tile_position=(64 * hh, 32 * h4))
                pmt = pm.tile([128, N], BF16, tag="pm")
                nc.scalar.activation(out=pmt, in_=st, func=AF.Exp, scale=QSC)
                pmems[g] = pmt

            def memsim_unit(qT, pmems, g):
                return lambda: memsim(qT, pmems, g)

            def attention_all(cfgs, fill):
                """All batches as one continuous round stream. cfg =
                (qT, vext, attn, pmems); batch b's sim rounds occupy global
                rounds 64b..64b+63, its av/epilogue work trails into the
                next batch's first rounds."""
                sts = [{"state": {}, "avbs": {}, "pts": [None] * 64} for _ in cfgs]

                def sim_round(ci, rr):
                    qT, vext, attn, pmems = cfgs[ci]
                    s, jc = rr // 8, rr % 8
                    p, h2 = s // 2, s % 2
                    st = sim_ps.tile([128, N], F32, tag="sim")
                    for hh in range(2):
                        nc.tensor.matmul(
                            st[:, hh * 512:(hh + 1) * 512],
                            kTp[64 * hh:64 * hh + 64, p, jc * 128:(jc + 1) * 128],
                            qT[64 * hh:64 * hh + 64, p, h2 * 512:(h2 + 1) * 512],
                            start=True, stop=True)
                    pt = pp.tile([128, N], BF16, tag="p")
                    nc.scalar.activation(out=pt, in_=st, func=AF.Exp, scale=QSC)
                    sts[ci]["pts"][rr] = pt

                def av_round(ci, rr):
                    qT, vext, attn, pmems = cfgs[ci]
                    s, jc = rr // 8, rr % 8
                    p, h2 = s // 2, s % 2
                    if jc == 0:
                        avA = av_ps.tile([65, 512], F32, tag="av")
                        avB = av_ps.tile([65, 512], F32, tag="av")
                        sts[ci]["state"][s] = (avA, avB)
                    avt = sts[ci]["state"][s]
                    for hh in range(2):
                        h = 2 * p + hh
                        nc.tensor.matmul(
                            avt[hh], vext[:, jc, h * (DH + 1):(h + 1) * (DH + 1)],
                            sts[ci]["pts"][rr][:, hh * 512:(hh + 1) * 512],
                            start=(jc == 0), stop=(jc == 7))
                    if jc == 5:
                        for hh in range(2):
                            h = 2 * p + hh
                            g, r0 = h // 4, 32 * (h % 4)
                            nc.tensor.matmul(
                                avt[hh],
                                vmem[r0:r0 + NMEM, g, (h % 4) * (DH + 1):(h % 4 + 1) * (DH + 1)],
                                pmems[g][r0:r0 + NMEM, h2 * 512:(h2 + 1) * 512],
                                start=False, stop=False, tile_position=(r0, 0))

                def epilogue_a(ci, s):
                    avt = sts[ci]["state"].pop(s)
                    pair = []
                    for hh in range(2):
                        avb = avsp.tile([65, 512], F32R, tag="avs")
                        with tc.high_priority(offset=64):
                            nc.vector.tensor_copy(out=avb, in_=avt[hh])
                        pair.append(avb)
                    sts[ci]["avbs"][s] = pair

                def epilogue_b(ci, s):
                    qT, vext, attn, pmems = cfgs[ci]
                    p, h2 = s // 2, s % 2
                    for hh in range(2):
                        avb = sts[ci]["avbs"][s][hh]
                        bc = qkv_ps.tile([64, 512], F32, tag="q")
                        nc.tensor.matmul(bc, ones1[64:65, :], avb[64:65, :], start=True, stop=True)
                        rcp = rp.tile([64, 512], F32, tag="rcp")
                        nc.vector.reciprocal_approx_fast(out=rcp, in_=bc)
                        nc.vector.tensor_mul(
                            out=attn[64 * hh:64 * hh + 64, p, h2 * 512:(h2 + 1) * 512],
                            in0=avb[0:64, :].bitcast(F32), in1=rcp)

                total = 64 * len(cfgs)
                for r in range(total + 13):
                    if r < total:
                        sim_round(r // 64, r % 64)
                    for ci in range(len(cfgs)):
                        rb = r - 64 * ci
                        for s in range(8):
                            o = rb - 8 * s
                            if o == 3:
                                av_round(ci, 8 * s + 0)
                                av_round(ci, 8 * s + 1)
                            elif 4 <= o <= 8:
                                av_round(ci, 8 * s + o - 2)
                            elif o == 10:
                                av_round(ci, 8 * s + 7)
                                epilogue_a(ci, s)
                            elif o == 12:
                                epilogue_b(ci, s)
                    for u in (fill[r] if r < len(fill) else []):
                        u()

            # ---------------- schedule ----------------
            # Prologue: only what rounds 0-2 of batch-0 attention need.
            xn0, n0_units = norm(0)
            for u in n0_units[:9]:           # xsq, ss, sqrt+recip
                u()
            qT0 = qp.tile([128, CT, N], BF16, tag="qT")
            qT1 = qp.tile([128, CT, N], BF16, tag="qT")
            for u in n0_units[9:13]:         # xn h2=0 halves
                u()
            qkproj(xn0, qT0, [0, 4], h2s=(0,))   # q/k pair 0, first i-half
            vproj(xn0, vexts[0], [0])
            for u in n0_units[13:17]:        # xn h2=1 halves
                u()
            qkproj(xn0, qT0, [0, 4], h2s=(1,))
            pmem0 = [None, None]
            xn1, n1_units = norm(1)
            for u in n1_units[:9]:           # norm-1 stats in prologue: the
                u()                          # SQRT table loads once, no
                                             # mid-stream table swaps
            # keep-alive bridge: dependency-free matmuls that hold the PE
            # HAM clock gate open between the ss chains and the projection
            # chains (a >3.4us PE-idle gap would re-throttle to 1.2 GHz)
            warm2 = qkv_ps.tile([128, 512], F32, tag="q")
            for _ in range(24):
                nc.tensor.matmul(warm2[:, 0:128], ones128, ones128,
                                 start=True, stop=True)
            nc.vector.tensor_copy(out=gsb, in_=warm2[0:128, 0:CT])

            attn0 = atp.tile([128, CT, N], BF16, tag="attn")
            attn1 = atp.tile([128, CT, N], BF16, tag="attn")
            pmem1 = [None, None]

            def place(fill, r0, units, per_round=1):
                r, i = r0, 0
                while i < len(units):
                    for _ in range(per_round):
                        if i < len(units):
                            fill[r].append(units[i])
                            i += 1
                    r += 1

            # merged fill plan over 141 rounds (batch-1 sims start at 64).
            # Issue-order deadlines: vext[jc]/pmem writes before their av
            # rounds; kTp pair i of batch b last read at 64b+8*(2i+1)+7.
            fl = [[] for _ in range(141)]
            q1u = qkproj_units(xn0, qT0, 1)
            v_units = {ic: vproj_units(xn0, vexts[0], ic) for ic in range(1, 8)}
            place(fl, 0, q1u, per_round=2)                         # q1, rounds 0-1
            place(fl, 2, [memsim_unit(qT0, pmem0, 0)] + v_units[1], per_round=3)
            place(fl, 3, v_units[2], per_round=2)
            place(fl, 4, v_units[3], per_round=2)
            place(fl, 5, v_units[4], per_round=2)
            place(fl, 6, v_units[5] + v_units[6], per_round=4)
            place(fl, 7, v_units[7], per_round=2)                  # v7 before round 8
            place(fl, 8, qkproj_units(xn0, qT0, 5))                # k1, read rnd 16
            place(fl, 12, qkproj_units(xn0, qT0, 6))               # k2, read rnd 32
            place(fl, 16, qkproj_units(xn0, qT0, 7))               # k3, read rnd 48
            place(fl, 20, qkproj_units(xn0, qT0, 2))               # q2
            place(fl, 24, qkproj_units(xn0, qT0, 3))               # q3
            place(fl, 28, [memsim_unit(qT0, pmem0, 1)])            # by round 39
            place(fl, 29, n1_units[9:13])                          # xn1 chunks
            place(fl, 33, qkproj_units(xn1, qT1, 0))
            place(fl, 37, qkproj_units(xn1, qT1, 1))
            place(fl, 41, [memsim_unit(qT1, pmem1, 0)])            # after q0',q1'
            place(fl, 42, qkproj_units(xn1, qT1, 4))               # batch-1 k0
            place(fl, 46, qkproj_units(xn1, qT1, 2))
            place(fl, 50, qkproj_units(xn1, qT1, 3))
            place(fl, 54, [memsim_unit(qT1, pmem1, 1)])
            place(fl, 55, [u for ic in range(8) for u in vproj_units(xn1, vexts[1], ic)],
                  per_round=2)                                     # rounds 55-62
            place(fl, 64 + 2, qkproj_units(xn1, qT1, 5))           # k1', read rnd 80
            place(fl, 64 + 6, qkproj_units(xn1, qT1, 6))           # k2', read rnd 96
            place(fl, 64 + 10, qkproj_units(xn1, qT1, 7))          # k3', read rnd 112
            place(fl, 64 + 14, [u for mc in range(CT) for h2 in range(2)
                                for u in proj_units(attn0, 0, mc, h2)])
            attention_all([(qT0, vexts[0], attn0, pmem0),
                           (qT1, vexts[1], attn1, pmem1)], fl)
            proj(attn1, 1)
    nc.compile()
    return nc


_NC_CACHE = []


def kernel(x, gamma, mem_kv, w_qkv, w_out, _trace=False):
    x = np.asarray(x, dtype=np.float32)
    gamma = np.asarray(gamma, dtype=np.float32)
    mem_kv = np.asarray(mem_kv, dtype=np.float32)
    w_qkv = np.asarray(w_qkv, dtype=np.float32)
    w_out = np.asarray(w_out, dtype=np.float32)

    b, c, hh, ww = x.shape
    n = hh * ww
    xs = x.reshape(b, c, n)

    wqkvt = np.ascontiguousarray(w_qkv.T).astype(ml_dtypes.bfloat16)  # [c, 3c]
    wot = np.ascontiguousarray(w_out.T).astype(ml_dtypes.bfloat16)     # [c, c]
    gammat = np.ascontiguousarray(gamma.reshape(CT, 128).T)  # [128, CT]

    memk = np.zeros((128, NPAIR, NMEM), np.float32)
    memv = np.zeros((128, 2, VW), np.float32)
    for h in range(HEADS):
        p, hh_ = h // 2, h % 2
        memk[64 * hh_:64 * hh_ + DH, p, 0:NMEM] = mem_kv[0, h].T  # [dh, nmem]
        g, r1, c0 = h // 4, 32 * (h % 4), (h % 4) * (DH + 1)
        memv[r1:r1 + NMEM, g, c0:c0 + DH] = mem_kv[1, h]
        memv[r1:r1 + NMEM, g, c0 + DH] = 1.0

    if not _NC_CACHE:
        _NC_CACHE.append(_build())
    nc = _NC_CACHE[0]

    in_maps = []
    for core in range(NCORES):
        in_maps.append({
            "x": np.ascontiguousarray(xs[core * PB:(core + 1) * PB]).astype(ml_dtypes.bfloat16),
            "wqkvt": wqkvt,
            "wot": wot,
            "gammat": gammat,
            "memk": memk,
            "memv": memv,
        })
    res = run_bass_kernel_spmd(nc, in_maps, core_ids=list(range(NCORES)), trace=_trace)
    out = np.concatenate([np.asarray(res.results[core]["out"], dtype=np.float32)
                          for core in range(NCORES)], axis=0)
    kernel.last_result = res
    return out.reshape(b, c, hh, ww)
